# revision 4
# baseline (speedup 1.0000x reference)
"""GCN (2-layer, PyG GCNConv-style) Trainium2 Bass kernel, SPMD over 8 NeuronCores.

Sharding: nodes are partitioned contiguously across the 8 cores (12544 each,
padded to 100352). Edges live on the core that owns their *destination* node.
Per layer: each core computes its slice of the dense transform, the slices are
AllGathered into a full per-node table in HBM, and each core aggregates its
incoming edges by gathering source rows (indirect DMA, 128 rows per call) and
scatter-adding them with one-hot matmuls into PSUM per 128-destination block.
"""

import numpy as np

N_NODES = 100000
N_EDGES = 1600000
D_FEAT = 512
HIDDEN = 64
N_CLASS = 16
DROP_P = 0.5

NCORE = 8
P = 128
NPAD = 100352            # 8 * 12544
PC = NPAD // NCORE       # 12544 nodes per core
NB = PC // P             # 98 dst blocks per core
CHUNKS = 19              # 128-edge chunks per dst block (fixed, data-independent)
NSLOT = NB * CHUNKS      # chunk-columns per core

_PROGRAM_CACHE = {}


def _dropout_scale_mask():
    """Reproduce the reference's fixed dropout mask: bernoulli(key(42), 0.5)."""
    import jax

    cpu = jax.devices("cpu")[0]
    with jax.default_device(cpu):
        keep = jax.random.bernoulli(jax.random.key(42), 1.0 - DROP_P, (N_NODES, HIDDEN))
        mask = np.asarray(keep, dtype=np.float32) * (1.0 / (1.0 - DROP_P))
    out = np.zeros((NPAD, HIDDEN), dtype=np.float32)
    out[:N_NODES] = mask
    return out


def _split_wide_waits(nc, max_waits=1):
    """This walrus build encodes at most one sync-wait per instruction; hoist
    excess waits onto preceding same-engine NoOps."""
    import bass_rust
    import concourse.mybir as mybir

    for f in nc.m.functions:
        for bb in f.blocks:
            out = []
            changed = False
            for inst in bb.instructions:
                si = inst.sync_info
                if si is not None and si.on_wait is not None and len(si.on_wait) > max_waits:
                    waits = list(si.on_wait)
                    keep, excess = waits[-max_waits:], waits[:-max_waits]
                    k = 0
                    while excess:
                        chunk, excess = excess[:max_waits], excess[max_waits:]
                        nop = bass_rust.InstNoOp(name=f"{inst.name}-wsplit{k}")
                        nop.engine = inst.engine
                        nop.sync_info = mybir.SyncInfo(on_wait=chunk, on_update=[])
                        out.append(nop)
                        k += 1
                    inst.sync_info = mybir.SyncInfo(on_wait=keep, on_update=list(si.on_update))
                    changed = True
                out.append(inst)
            if changed:
                bb.instructions = out


def _build_program():
    import concourse.bass as bass
    import concourse.tile as tile
    from concourse import mybir

    f32 = mybir.dt.float32
    i32 = mybir.dt.int32

    nc = bass.Bass("TRN2", target_bir_lowering=False, debug=False, num_devices=NCORE)

    xT = nc.declare_dram_parameter("xT", [D_FEAT, PC], f32, isOutput=False)
    W1 = nc.declare_dram_parameter("W1", [D_FEAT, HIDDEN], f32, isOutput=False)
    W2 = nc.declare_dram_parameter("W2", [HIDDEN, N_CLASS], f32, isOutput=False)
    b1 = nc.declare_dram_parameter("b1", [1, HIDDEN], f32, isOutput=False)
    b2 = nc.declare_dram_parameter("b2", [1, N_CLASS], f32, isOutput=False)
    maskin = nc.declare_dram_parameter("mask", [PC, HIDDEN], f32, isOutput=False)
    iotain = nc.declare_dram_parameter("iota", [P, P], f32, isOutput=False)
    identin = nc.declare_dram_parameter("ident", [P, P], f32, isOutput=False)
    eidx = nc.declare_dram_parameter("eidx", [P, NSLOT], i32, isOutput=False)
    edst = nc.declare_dram_parameter("edst", [P, NSLOT], f32, isOutput=False)
    enrm = nc.declare_dram_parameter("enrm", [P, NSLOT], f32, isOutput=False)
    out_ls = nc.declare_dram_parameter("out_ls", [PC, N_CLASS], f32, isOutput=True)
    out_xo = nc.declare_dram_parameter("out_xo", [PC, N_CLASS], f32, isOutput=True)

    # internal DRAM: per-core shard + allgathered full tables
    h1shard = nc.dram_tensor("h1shard", [PC, HIDDEN], f32)
    h2shard = nc.dram_tensor("h2shard", [PC, HIDDEN], f32)
    H1 = nc.dram_tensor("H1", [NPAD, HIDDEN], f32, addr_space="Shared")
    H2 = nc.dram_tensor("H2", [NPAD, HIDDEN], f32, addr_space="Shared")

    groups = [list(range(NCORE))]

    with tile.TileContext(nc) as tc:
        with (
            tc.tile_pool(name="cst", bufs=1) as cst,
            tc.tile_pool(name="sb", bufs=6) as sb,
            tc.tile_pool(name="ps", bufs=3, space="PSUM") as ps,
            tc.tile_pool(name="ps2", bufs=2, space="PSUM") as ps2,
        ):
            # resident constants
            w1sb = cst.tile([P, 4 * HIDDEN], f32)
            for k in range(4):
                nc.sync.dma_start(out=w1sb[:, k * HIDDEN:(k + 1) * HIDDEN],
                                  in_=W1[k * P:(k + 1) * P, :])
            w2sb = cst.tile([HIDDEN, N_CLASS], f32)
            nc.sync.dma_start(out=w2sb[:], in_=W2[:, :])
            b1sb = cst.tile([1, HIDDEN], f32)
            nc.sync.dma_start(out=b1sb[:], in_=b1[:, :])
            b2sb = cst.tile([1, N_CLASS], f32)
            nc.sync.dma_start(out=b2sb[:], in_=b2[:, :])
            onesb = cst.tile([1, P], f32)
            nc.vector.memset(onesb[:], 1.0)
            iotasb = cst.tile([P, P], f32)
            nc.sync.dma_start(out=iotasb[:], in_=iotain[:, :])
            identsb = cst.tile([P, P], f32)
            nc.sync.dma_start(out=identsb[:], in_=identin[:, :])
            masksb = cst.tile([P, NB * HIDDEN], f32)
            # mask[pc, f] -> masksb[pc % 128, (pc // 128) * 64 + f]
            mask_dram_view = bass.AP(
                maskin[:, :].tensor, maskin[:, :].offset,
                [[HIDDEN, P], [P * HIDDEN, NB], [1, HIDDEN]])
            nc.sync.dma_start(out=masksb[:], in_=mask_dram_view)
            eidxsb = cst.tile([P, NSLOT], i32)
            nc.sync.dma_start(out=eidxsb[:], in_=eidx[:, :])
            edstsb = cst.tile([P, NSLOT], f32)
            nc.sync.dma_start(out=edstsb[:], in_=edst[:, :])
            enrmsb = cst.tile([P, NSLOT], f32)
            nc.sync.dma_start(out=enrmsb[:], in_=enrm[:, :])
            xosb = cst.tile([P, NB * N_CLASS], f32)

            # ---- phase 1: h1 = x @ W1 (per-core rows), allgather -> H1 ----
            for b in range(NB):
                acc = ps.tile([P, HIDDEN], f32, tag="acc")
                for k in range(4):
                    xt = sb.tile([P, P], f32, tag="xt")
                    nc.sync.dma_start(out=xt[:], in_=xT[k * P:(k + 1) * P, b * P:(b + 1) * P])
                    nc.tensor.matmul(out=acc[:], lhsT=xt[:],
                                     rhs=w1sb[:, k * HIDDEN:(k + 1) * HIDDEN],
                                     start=(k == 0), stop=(k == 3))
                h = sb.tile([P, HIDDEN], f32, tag="h1o")
                nc.vector.tensor_copy(out=h[:], in_=acc[:])
                nc.sync.dma_start(out=h1shard[b * P:(b + 1) * P, :], in_=h[:])
            nc.gpsimd.collective_compute(
                "AllGather", mybir.AluOpType.bypass, replica_groups=groups,
                ins=[h1shard[:, :]], outs=[H1[:, :]])

            # ---- phase 2: aggregate layer 1, relu+bias+dropout -> h2shard, allgather ----
            def aggregate(table, b, seed_bias):
                acc = ps.tile([P, HIDDEN], f32, tag="acc")
                if seed_bias is not None:
                    nc.tensor.matmul(out=acc[:], lhsT=onesb[:], rhs=seed_bias,
                                     start=True, stop=False)
                for c in range(CHUNKS):
                    j = b * CHUNKS + c
                    m = sb.tile([P, HIDDEN], f32, tag="msg")
                    nc.gpsimd.indirect_dma_start(
                        out=m[:], out_offset=None, in_=table[:, :],
                        in_offset=bass.IndirectOffsetOnAxis(ap=eidxsb[:, j:j + 1], axis=0))
                    oh = sb.tile([P, P], f32, tag="oh")
                    nc.vector.tensor_scalar(
                        out=oh[:], in0=iotasb[:],
                        scalar1=edstsb[:, j:j + 1], scalar2=enrmsb[:, j:j + 1],
                        op0=mybir.AluOpType.is_equal, op1=mybir.AluOpType.mult)
                    nc.tensor.matmul(out=acc[:], lhsT=oh[:], rhs=m[:],
                                     start=(seed_bias is None and c == 0),
                                     stop=(c == CHUNKS - 1))
                return acc

            for b in range(NB):
                acc = aggregate(H1, b, b1sb[:])
                a = sb.tile([P, HIDDEN], f32, tag="h2o")
                nc.scalar.activation(a[:], acc[:], mybir.ActivationFunctionType.Relu)
                nc.vector.tensor_tensor(
                    out=a[:], in0=a[:], in1=masksb[:, b * HIDDEN:(b + 1) * HIDDEN],
                    op=mybir.AluOpType.mult)
                nc.sync.dma_start(out=h2shard[b * P:(b + 1) * P, :], in_=a[:])
            nc.gpsimd.collective_compute(
                "AllGather", mybir.AluOpType.bypass, replica_groups=groups,
                ins=[h2shard[:, :]], outs=[H2[:, :]])

            # ---- phase 3: aggregate layer 2 (width 64), then @W2 + b2 ----
            for b in range(NB):
                acc = aggregate(H2, b, None)
                g = sb.tile([P, HIDDEN], f32, tag="g2")
                nc.vector.tensor_copy(out=g[:], in_=acc[:])
                tps = ps2.tile([HIDDEN, P], f32, tag="tp")
                nc.tensor.transpose(out=tps[:], in_=g[:], identity=identsb[:])
                gT = sb.tile([HIDDEN, P], f32, tag="gT")
                nc.vector.tensor_copy(out=gT[:], in_=tps[:])
                xo = ps2.tile([P, N_CLASS], f32, tag="xo")
                nc.tensor.matmul(out=xo[:], lhsT=onesb[:], rhs=b2sb[:],
                                 start=True, stop=False)
                nc.tensor.matmul(out=xo[:], lhsT=gT[:], rhs=w2sb[:],
                                 start=False, stop=True)
                nc.vector.tensor_copy(out=xosb[:, b * N_CLASS:(b + 1) * N_CLASS], in_=xo[:])

            # ---- phase 4: batched log_softmax over [P, NB*16] ----
            def view3(t, inner):
                a = t[:]
                return bass.AP(a.tensor, a.offset, [a.ap[0], [inner, NB], [1, inner]])

            def bcast(t, inner):
                a = t[:]
                return bass.AP(a.tensor, a.offset, [a.ap[0], [1, NB], [0, inner]])

            mx = cst.tile([P, NB], f32)
            nc.vector.tensor_reduce(out=mx[:], in_=view3(xosb, N_CLASS),
                                    axis=mybir.AxisListType.X, op=mybir.AluOpType.max)
            sh = cst.tile([P, NB * N_CLASS], f32)
            nc.vector.tensor_tensor(out=sh[:], in0=xosb[:], in1=bcast(mx, N_CLASS),
                                    op=mybir.AluOpType.subtract)
            ex = cst.tile([P, NB * N_CLASS], f32)
            nc.scalar.activation(ex[:], sh[:], mybir.ActivationFunctionType.Exp)
            sm = cst.tile([P, NB], f32)
            nc.vector.tensor_reduce(out=sm[:], in_=view3(ex, N_CLASS),
                                    axis=mybir.AxisListType.X, op=mybir.AluOpType.add)
            ln = cst.tile([P, NB], f32)
            nc.scalar.activation(ln[:], sm[:], mybir.ActivationFunctionType.Ln)
            ls = cst.tile([P, NB * N_CLASS], f32)
            nc.vector.tensor_tensor(out=ls[:], in0=sh[:], in1=bcast(ln, N_CLASS),
                                    op=mybir.AluOpType.subtract)

            # outputs: sbuf [p, (b, f)] -> dram [(b*128+p), f]
            def out_view(t):
                a = t[:, :]
                return bass.AP(a.tensor, a.offset,
                               [[N_CLASS, P], [P * N_CLASS, NB], [1, N_CLASS]])

            nc.sync.dma_start(out=out_view(out_xo), in_=xosb[:])
            nc.sync.dma_start(out=out_view(out_ls), in_=ls[:])

    _split_wide_waits(nc)
    return nc


def _preprocess(x, edge_index, e_w):
    """Degrees, norms, per-core dst-sorted edge slots, transposed x."""
    src = edge_index[0].astype(np.int64)
    dst = edge_index[1].astype(np.int64)
    w = e_w.astype(np.float64)
    loops = np.arange(N_NODES, dtype=np.int64)
    src_f = np.concatenate([src, loops])
    dst_f = np.concatenate([dst, loops])
    w_f = np.concatenate([w, np.ones(N_NODES)])

    deg = np.bincount(dst_f, weights=w_f, minlength=N_NODES)
    dis = np.where(deg > 0, 1.0 / np.sqrt(np.maximum(deg, 1e-12)), 0.0)
    norm = (dis[src_f] * w_f * dis[dst_f]).astype(np.float32)

    # bucket edges by core / dst block
    order = np.argsort(dst_f, kind="stable")
    src_s = src_f[order].astype(np.int32)
    dst_s = dst_f[order].astype(np.int32)
    nrm_s = norm[order]

    blk = dst_s >> 7                      # global 128-dst block id (0 .. NPAD/128)
    nblk_tot = NPAD // P
    counts = np.bincount(blk, minlength=nblk_tot)
    assert counts.max() <= CHUNKS * P, f"block overflow: {counts.max()}"
    starts = np.zeros(nblk_tot + 1, dtype=np.int64)
    np.cumsum(counts, out=starts[1:])

    # slot arrays per core: [P, NSLOT]
    eidx = np.zeros((NCORE, P, NSLOT), dtype=np.int32)
    edst = np.zeros((NCORE, P, NSLOT), dtype=np.float32)
    enrm = np.zeros((NCORE, P, NSLOT), dtype=np.float32)

    # position of each edge within its block
    pos_in_blk = np.arange(len(dst_s)) - starts[blk]
    p_of = (pos_in_blk % P).astype(np.int64)
    c_of = (pos_in_blk // P).astype(np.int64)
    core_of = blk // NB
    b_of = blk % NB
    col = b_of * CHUNKS + c_of
    eidx[core_of, p_of, col] = src_s
    edst[core_of, p_of, col] = (dst_s & 127).astype(np.float32)
    enrm[core_of, p_of, col] = nrm_s

    xp = np.zeros((NPAD, D_FEAT), dtype=np.float32)
    xp[:N_NODES] = x
    xTs = [np.ascontiguousarray(xp[c * PC:(c + 1) * PC].T) for c in range(NCORE)]
    return xTs, eidx, edst, enrm


def kernel(x, edge_index, e_w, W1, b1, W2, b2):
    from concourse.bass_utils import run_bass_kernel_spmd

    if "prog" not in _PROGRAM_CACHE:
        _PROGRAM_CACHE["prog"] = _build_program()
    nc = _PROGRAM_CACHE["prog"]

    xTs, eidx, edst, enrm = _preprocess(x, edge_index, e_w)
    mask = _dropout_scale_mask()
    iota = np.ascontiguousarray(np.broadcast_to(
        np.arange(P, dtype=np.float32), (P, P)))
    ident = np.eye(P, dtype=np.float32)

    in_maps = []
    for c in range(NCORE):
        in_maps.append({
            "xT": xTs[c],
            "W1": np.ascontiguousarray(W1, dtype=np.float32),
            "W2": np.ascontiguousarray(W2, dtype=np.float32),
            "b1": np.ascontiguousarray(b1, dtype=np.float32).reshape(1, HIDDEN),
            "b2": np.ascontiguousarray(b2, dtype=np.float32).reshape(1, N_CLASS),
            "mask": np.ascontiguousarray(mask[c * PC:(c + 1) * PC]),
            "iota": iota,
            "ident": ident,
            "eidx": np.ascontiguousarray(eidx[c]),
            "edst": np.ascontiguousarray(edst[c]),
            "enrm": np.ascontiguousarray(enrm[c]),
        })

    res = run_bass_kernel_spmd(nc, in_maps, list(range(NCORE)))
    ls = np.concatenate([res.results[c]["out_ls"] for c in range(NCORE)], axis=0)
    xo = np.concatenate([res.results[c]["out_xo"] for c in range(NCORE)], axis=0)
    return ls[:N_NODES], xo[:N_NODES]
